# revision 1
# baseline (speedup 1.0000x reference)
"""Distributed 3-layer GraphConv GNN on 8 Trainium2 NeuronCores.

Sharding: nodes (and their incoming edges) are partitioned contiguously
across the 8 cores (2560 nodes / 20 blocks of 128 dst nodes per core).
Per layer, each core:
  - dma_gathers the source-node feature rows for its edges (sorted by dst,
    padded to a uniform chunk count so the SPMD program is identical on
    every core),
  - segment-sums them per 128-dst block on the TensorEngine via one-hot
    matmuls (one-hot built on the VectorEngine with is_equal against an
    iota constant),
  - applies the dense W_rel/W_root transform + bias + ReLU,
  - transposes to node-major and AllGathers the full feature matrix so the
    next layer can gather from it.
Graph pooling is a segment-sum over the (sorted) batch vector done locally
with the same one-hot matmul trick, AllReduced across cores, followed by
the output projection and log_softmax (all computed redundantly per core).
"""
import sys

sys.path.insert(0, "/opt/trn_rl_repo")

from contextlib import ExitStack

import numpy as np

import concourse.bass as bass
import concourse.tile as tile
from concourse import bacc, mybir
from concourse.bass_utils import run_bass_kernel_spmd
from concourse.library_config import mlp as mlp_lib

N, E, F_IN, H, C_OUT, G = 20000, 640000, 64, 128, 10, 128
NCORES = 8
NPC = 2560          # nodes per core
NBLK = NPC // 128   # dst blocks per core (20)
NPAD = NCORES * NPC  # 20480
F32 = mybir.dt.float32
AF = mybir.ActivationFunctionType
ALU = mybir.AluOpType


def _prep_inputs(x, edge_index, batch):
    """Host-side edge partitioning/padding. Returns per-core input dicts
    (minus weights) and the uniform chunks-per-block count."""
    src = np.asarray(edge_index[0], dtype=np.int64)
    dst = np.asarray(edge_index[1], dtype=np.int64)
    batch = np.asarray(batch, dtype=np.int64)
    x = np.ascontiguousarray(np.asarray(x, dtype=np.float32))

    order = np.argsort(dst, kind="stable")
    dst_s = dst[order]
    src_s = src[order]
    nblk_glob = NCORES * NBLK  # 160 (157 contain real nodes)
    starts = np.searchsorted(dst_s, np.arange(nblk_glob) * 128)
    ends = np.searchsorted(dst_s, (np.arange(nblk_glob) + 1) * 128)
    counts = ends - starts
    cchunks = max(1, int(np.ceil(counts.max() / 128)))
    L = cchunks * 128  # padded edges per block

    iota = np.tile(np.arange(128, dtype=np.float32), (128, 1))
    ident = np.eye(128, dtype=np.float32)
    ones_row = np.ones((1, 128), np.float32)

    in_maps = []
    for k in range(NCORES):
        src_pad = np.zeros((NBLK, L), np.int64)
        dstrel_pad = np.full((NBLK, L), -1.0, np.float32)
        for b in range(NBLK):
            gb = k * NBLK + b
            s, e = starts[gb], ends[gb]
            n = e - s
            if n:
                src_pad[b, :n] = src_s[s:e]
                dstrel_pad[b, :n] = (dst_s[s:e] - gb * 128).astype(np.float32)
        # dma_gather idx layout: idx i at [i % 16, i // 16], replicated
        # across the 8 groups of 16 partitions.
        idx16 = src_pad.reshape(NBLK, L // 16, 16).transpose(0, 2, 1)
        idx_t = np.concatenate(list(np.tile(idx16, (1, 8, 1))), axis=1)
        idx_t = idx_t.astype(np.int16)
        # dst_rel layout: edge e = c*128 + p at [p, c]
        dr = dstrel_pad.reshape(NBLK, cchunks, 128).transpose(0, 2, 1)
        dr_t = np.ascontiguousarray(np.concatenate(list(dr), axis=1))

        # graph id per local node, [128, NBLK]; -1 for pad nodes
        gids = np.full((NBLK, 128), -1.0, np.float32)
        base = k * NPC
        valid = max(0, min(NPC, N - base))
        if valid:
            flat = np.full(NPC, -1.0, np.float32)
            flat[:valid] = batch[base : base + valid].astype(np.float32)
            gids = flat.reshape(NBLK, 128)
        batchrel_t = np.ascontiguousarray(gids.T)  # [128, NBLK]

        xT = np.zeros((F_IN, NPC), np.float32)
        if valid:
            xT[:, :valid] = x[base : base + valid].T

        in_maps.append(
            {
                "x_full": x,
                "idx_t": idx_t,
                "dstrel_t": dr_t,
                "batchrel_t": batchrel_t,
                "xT_t": np.ascontiguousarray(xT),
                "iota_t": iota,
                "ident_t": ident,
                "ones_t": ones_row,
            }
        )
    return in_maps, cchunks


def _build_program(cchunks, active_blocks=NBLK, active_chunks=None):
    """active_blocks/active_chunks < full sizes build a truncated program
    (for bisection/debug only — output is numerically wrong)."""
    L = cchunks * 128
    a_blk = active_blocks
    a_chk = active_chunks or cchunks
    nc = bacc.Bacc("TRN2", target_bir_lowering=False, debug=False,
                   num_devices=NCORES)

    x_full = nc.dram_tensor("x_full", [N, F_IN], F32, kind="ExternalInput")
    idx_t = nc.dram_tensor("idx_t", [128, NBLK * L // 16], mybir.dt.int16,
                           kind="ExternalInput")
    dstrel_t = nc.dram_tensor("dstrel_t", [128, NBLK * cchunks], F32,
                              kind="ExternalInput")
    batchrel_t = nc.dram_tensor("batchrel_t", [128, NBLK], F32,
                                kind="ExternalInput")
    xT_t = nc.dram_tensor("xT_t", [F_IN, NPC], F32, kind="ExternalInput")
    iota_t = nc.dram_tensor("iota_t", [128, 128], F32, kind="ExternalInput")
    ident_t = nc.dram_tensor("ident_t", [128, 128], F32, kind="ExternalInput")
    ones_t = nc.dram_tensor("ones_t", [1, 128], F32, kind="ExternalInput")
    w_rel_in = [nc.dram_tensor(f"w{i}_rel", [F_IN if i == 1 else H, H], F32,
                               kind="ExternalInput") for i in (1, 2, 3)]
    w_root_in = [nc.dram_tensor(f"w{i}_root", [F_IN if i == 1 else H, H], F32,
                                kind="ExternalInput") for i in (1, 2, 3)]
    b_in = [nc.dram_tensor(f"b{i}", [H, 1], F32, kind="ExternalInput")
            for i in (1, 2, 3)]
    w_out_in = nc.dram_tensor("w_out", [H, C_OUT], F32, kind="ExternalInput")
    b_out_in = nc.dram_tensor("b_out", [1, C_OUT], F32, kind="ExternalInput")
    out_t = nc.dram_tensor("out", [G, C_OUT], F32, kind="ExternalOutput")

    with tile.TileContext(nc) as tc, ExitStack() as ctx:
        const = ctx.enter_context(tc.tile_pool(name="const", bufs=1))
        feat = ctx.enter_context(tc.tile_pool(name="feat", bufs=1))
        xe_pool = ctx.enter_context(tc.tile_pool(name="xe", bufs=3))
        m_pool = ctx.enter_context(tc.tile_pool(name="m", bufs=4))
        nm_pool = ctx.enter_context(tc.tile_pool(name="nm", bufs=3))
        sm_pool = ctx.enter_context(tc.tile_pool(name="sm", bufs=1))
        psA = ctx.enter_context(tc.tile_pool(name="psA", bufs=2, space="PSUM"))
        psB = ctx.enter_context(tc.tile_pool(name="psB", bufs=2, space="PSUM"))
        psT = ctx.enter_context(tc.tile_pool(name="psT", bufs=2, space="PSUM"))
        psP = ctx.enter_context(tc.tile_pool(name="psP", bufs=1, space="PSUM"))
        dram = ctx.enter_context(tc.tile_pool(name="dram", bufs=1, space="DRAM"))

        nc.gpsimd.load_library(mlp_lib)

        def load_const(name, dram_h, shape, dtype=F32):
            t = const.tile(shape, dtype, name=name)
            nc.sync.dma_start(t[:], dram_h[:].ap() if hasattr(dram_h, "ap") else dram_h[:])
            return t

        idx_sb = const.tile([128, NBLK * L // 16], mybir.dt.int16)
        nc.sync.dma_start(idx_sb[:], idx_t[:])
        dstrel_sb = const.tile([128, NBLK * cchunks], F32)
        nc.sync.dma_start(dstrel_sb[:], dstrel_t[:])
        batchrel_sb = const.tile([128, NBLK], F32)
        nc.sync.dma_start(batchrel_sb[:], batchrel_t[:])
        iota_sb = const.tile([128, 128], F32)
        nc.sync.dma_start(iota_sb[:], iota_t[:])
        ident_sb = const.tile([128, 128], F32)
        nc.sync.dma_start(ident_sb[:], ident_t[:])
        ones_sb = const.tile([1, 128], F32)
        nc.sync.dma_start(ones_sb[:], ones_t[:])
        w_rel_sb, w_root_sb, b_sb = [], [], []
        for i in range(3):
            fi = F_IN if i == 0 else H
            wr = const.tile([fi, H], F32, name=f"wrel{i}")
            nc.sync.dma_start(wr[:], w_rel_in[i][:])
            w_rel_sb.append(wr)
            wo = const.tile([fi, H], F32, name=f"wroot{i}")
            nc.sync.dma_start(wo[:], w_root_in[i][:])
            w_root_sb.append(wo)
            bb = const.tile([H, 1], F32, name=f"b{i}")
            nc.sync.dma_start(bb[:], b_in[i][:])
            b_sb.append(bb)
        wout_sb = const.tile([H, C_OUT], F32)
        nc.sync.dma_start(wout_sb[:], w_out_in[:])
        bout_sb = const.tile([1, C_OUT], F32)
        nc.sync.dma_start(bout_sb[:], b_out_in[:])

        xT_sb = feat.tile([F_IN, NPC], F32)
        nc.sync.dma_start(xT_sb[:], xT_t[:])
        h1T_sb = feat.tile([H, NPC], F32)
        h2T_sb = feat.tile([H, NPC], F32)
        h3T_sb = feat.tile([H, NPC], F32)
        aggT_sb = feat.tile([H, NPC], F32)

        h1_loc = dram.tile([NPC, H], F32)
        h2_loc = dram.tile([NPC, H], F32)
        h1_full = dram.tile([NPAD, H], F32)
        h2_full = dram.tile([NPAD, H], F32)
        pool_in = dram.tile([H, G], F32)
        pool_out = dram.tile([H, G], F32)

        def gcn_layer(li, f_in, gather_src, inT_sb, outT_sb, h_loc, h_full):
            wrel, wroot, bb = w_rel_sb[li], w_root_sb[li], b_sb[li]
            for b in range(a_blk):
                xe = xe_pool.tile([128, cchunks * 128], F32, tag="xe",
                                  name=f"xe{li}_{b}")
                GCH = 8  # chunks per dma_gather call (1024 idxs max: larger
                         # single calls crash the device)
                for g0 in range(0, a_chk, GCH):
                    g1 = min(g0 + GCH, a_chk)
                    nsub = (g1 - g0) * 128
                    xe3 = xe[:, g0 * f_in : g1 * f_in].rearrange(
                        "p (c f) -> p c f", f=f_in)
                    nc.gpsimd.dma_gather(
                        xe3, gather_src[:],
                        idx_sb[:, b * (L // 16) + g0 * 8
                               : b * (L // 16) + g0 * 8 + nsub // 16],
                        nsub, nsub, f_in)
                agg_ps = psA.tile([128, 128], F32, tag="agg",
                                  name=f"agg{li}_{b}")
                for c in range(a_chk):
                    m = m_pool.tile([128, 128], F32, tag="m",
                                    name=f"m{li}_{b}_{c}")
                    nc.vector.tensor_scalar(
                        m[:], iota_sb[:],
                        dstrel_sb[:, b * cchunks + c : b * cchunks + c + 1],
                        None, ALU.is_equal)
                    nc.tensor.matmul(
                        agg_ps[:f_in, :],
                        xe[:, c * f_in : (c + 1) * f_in],
                        m[:],
                        start=(c == 0), stop=(c == a_chk - 1))
                nc.vector.tensor_copy(
                    aggT_sb[:f_in, b * 128 : (b + 1) * 128], agg_ps[:f_in, :])
            # dense transform + bias + relu (feature-major)
            for g in range(NPC // 512):
                hp = psB.tile([H, 512], F32, tag="hp", name=f"hp{li}_{g}")
                nc.tensor.matmul(hp[:], wrel[:],
                                 aggT_sb[:f_in, g * 512 : (g + 1) * 512],
                                 start=True, stop=False)
                nc.tensor.matmul(hp[:], wroot[:],
                                 inT_sb[:f_in, g * 512 : (g + 1) * 512],
                                 start=False, stop=True)
                nc.scalar.activation(outT_sb[:, g * 512 : (g + 1) * 512],
                                     hp[:], AF.Relu, bias=bb[:])
            # node-major store + allgather for next layer's gather source
            if h_loc is not None:
                for b in range(a_blk):
                    tp = psT.tile([128, 128], F32, tag="tp",
                                  name=f"tp{li}_{b}")
                    nc.tensor.transpose(
                        tp[:], outT_sb[:, b * 128 : (b + 1) * 128],
                        ident_sb[:])
                    nm = nm_pool.tile([128, 128], F32, tag="nm",
                                      name=f"nm{li}_{b}")
                    nc.scalar.copy(nm[:], tp[:])
                    nc.sync.dma_start(h_loc[b * 128 : (b + 1) * 128, :],
                                      nm[:])
                nc.gpsimd.collective_compute(
                    "AllGather", ALU.bypass,
                    replica_groups=[list(range(NCORES))],
                    ins=[h_loc.opt()], outs=[h_full.opt()])

        gcn_layer(0, F_IN, x_full, xT_sb, h1T_sb, h1_loc, h1_full)
        gcn_layer(1, H, h1_full, h1T_sb, h2T_sb, h2_loc, h2_full)
        gcn_layer(2, H, h2_full, h2T_sb, h3T_sb, None, None)

        # ---- pooling: pooledT[h, g] = sum_n h3[n, h] * (batch[n] == g) ----
        pool_ps = psP.tile([H, G], F32)
        for b in range(a_blk):
            tp = psT.tile([128, 128], F32, tag="tp", name=f"tpp_{b}")
            nc.tensor.transpose(tp[:], h3T_sb[:, b * 128 : (b + 1) * 128],
                                ident_sb[:])
            nm = nm_pool.tile([128, 128], F32, tag="nm", name=f"nmp_{b}")
            nc.scalar.copy(nm[:], tp[:])
            pb = m_pool.tile([128, 128], F32, tag="m", name=f"pb_{b}")
            nc.vector.tensor_scalar(pb[:], iota_sb[:],
                                    batchrel_sb[:, b : b + 1], None,
                                    ALU.is_equal)
            nc.tensor.matmul(pool_ps[:], nm[:], pb[:],
                             start=(b == 0), stop=(b == a_blk - 1))
        poolT_sb = sm_pool.tile([H, G], F32)
        nc.vector.tensor_copy(poolT_sb[:], pool_ps[:])
        nc.sync.dma_start(pool_in[:], poolT_sb[:])
        nc.gpsimd.collective_compute(
            "AllReduce", ALU.add, replica_groups=[list(range(NCORES))],
            ins=[pool_in.opt()], outs=[pool_out.opt()])
        poolT_full = sm_pool.tile([H, G], F32)
        nc.sync.dma_start(poolT_full[:], pool_out[:])

        # ---- logits = pooled @ w_out + b_out, then log_softmax ----
        log_ps = psB.tile([H, 512], F32, tag="hp", name="log_ps")
        nc.tensor.matmul(log_ps[:G, :C_OUT], poolT_full[:], wout_sb[:],
                         start=True, stop=False)
        nc.tensor.matmul(log_ps[:G, :C_OUT], ones_sb[:], bout_sb[:],
                         start=False, stop=True)
        logits = sm_pool.tile([G, C_OUT], F32)
        nc.vector.tensor_copy(logits[:], log_ps[:G, :C_OUT])
        mx = sm_pool.tile([G, 1], F32)
        nc.vector.tensor_reduce(mx[:], logits[:], mybir.AxisListType.X,
                                ALU.max)
        negmx = sm_pool.tile([G, 1], F32)
        nc.scalar.mul(negmx[:], mx[:], -1.0)
        expv = sm_pool.tile([G, C_OUT], F32)
        nc.scalar.activation(expv[:], logits[:], AF.Exp, bias=negmx[:])
        sm = sm_pool.tile([G, 1], F32)
        nc.vector.tensor_reduce(sm[:], expv[:], mybir.AxisListType.X, ALU.add)
        lse = sm_pool.tile([G, 1], F32)
        nc.scalar.activation(lse[:], sm[:], AF.Ln)
        mxlse = sm_pool.tile([G, 1], F32)
        nc.vector.tensor_add(mxlse[:], mx[:], lse[:])
        outv = sm_pool.tile([G, C_OUT], F32)
        nc.vector.tensor_scalar(outv[:], logits[:], mxlse[:], None,
                                ALU.subtract)
        nc.sync.dma_start(out_t[:], outv[:])

    nc.compile()
    return nc


_CACHE = {}


def kernel(x, edge_index, batch, w1_rel, b1, w1_root, w2_rel, b2, w2_root,
           w3_rel, b3, w3_root, w_out, b_out):
    in_maps, cchunks = _prep_inputs(x, edge_index, batch)
    weights = {
        "w1_rel": np.asarray(w1_rel, np.float32),
        "w1_root": np.asarray(w1_root, np.float32),
        "w2_rel": np.asarray(w2_rel, np.float32),
        "w2_root": np.asarray(w2_root, np.float32),
        "w3_rel": np.asarray(w3_rel, np.float32),
        "w3_root": np.asarray(w3_root, np.float32),
        "b1": np.asarray(b1, np.float32).reshape(H, 1),
        "b2": np.asarray(b2, np.float32).reshape(H, 1),
        "b3": np.asarray(b3, np.float32).reshape(H, 1),
        "w_out": np.asarray(w_out, np.float32),
        "b_out": np.asarray(b_out, np.float32).reshape(1, C_OUT),
    }
    for m in in_maps:
        m.update(weights)

    if cchunks not in _CACHE:
        _CACHE[cchunks] = _build_program(cchunks)
    nc = _CACHE[cchunks]
    res = run_bass_kernel_spmd(nc, in_maps, core_ids=list(range(NCORES)))
    return np.asarray(res.results[0]["out"], np.float32)



# revision 3
# speedup vs baseline: 1.4598x; 1.4598x over previous
"""Distributed 3-layer GraphConv GNN on 8 Trainium2 NeuronCores.

Sharding: nodes (and their incoming edges) are partitioned contiguously
across the 8 cores (2500 real nodes / 20 blocks of 128 padded dst rows per
core). The gather source (node features) lives in DRAM node-major in bf16
(gather cost is descriptor-bound, so bf16 halves SBUF/compute, not DMA).
Per layer, each core:
  - dma_gathers the source-node bf16 feature rows for its edges (sorted by
    dst, padded to a uniform chunk count so the SPMD program is identical
    on every core),
  - segment-sums them per 128-dst block on the TensorEngine via one-hot
    matmuls in bf16 (one-hot built on the VectorEngine with is_equal),
  - applies the dense W_rel (bf16) / W_root (fp32, reads the previous
    layer's resident feature-major activations) transform + bias + ReLU,
  - transposes its local output to node-major bf16 and AllGathers into the
    full gather-source matrix for the next layer (Shared DRAM output).
Graph pooling is a segment-sum over the (sorted) batch vector via the same
one-hot matmul trick in fp32, AllReduced across cores, followed by the
output projection and log_softmax (computed redundantly per core).
"""
import sys

sys.path.insert(0, "/opt/trn_rl_repo")

from contextlib import ExitStack

import numpy as np
import ml_dtypes

import concourse.bass as bass
import concourse.tile as tile
from concourse import bacc, mybir, library_config
from concourse.bass_utils import run_bass_kernel_spmd

N, E, F_IN, H, C_OUT, G = 20000, 640000, 64, 128, 10, 128
NCORES = 8
NPC = 2500            # real nodes per core
NBLK = 20             # dst blocks of 128 per core
NPCP = NBLK * 128     # padded rows per core (2560)
NPAD = NCORES * NPCP  # padded total rows (20480)
F32 = mybir.dt.float32
BF16 = mybir.dt.bfloat16
AF = mybir.ActivationFunctionType
ALU = mybir.AluOpType
BF = ml_dtypes.bfloat16


def _prep_inputs(x, edge_index, batch):
    """Host-side edge partitioning/padding. Returns per-core input dicts
    (minus weights) and the uniform chunks-per-block count."""
    src = np.asarray(edge_index[0], dtype=np.int64)
    dst = np.asarray(edge_index[1], dtype=np.int64)
    batch = np.asarray(batch, dtype=np.int64)
    x = np.ascontiguousarray(np.asarray(x, dtype=np.float32))

    order = np.argsort(dst, kind="stable")
    dst_s = dst[order]
    # remap src to padded node-major rows: core k slab at [k*2560, k*2560+2500)
    src_s = src[order]
    src_m = (src_s // NPC) * NPCP + (src_s % NPC)

    # per-core-block boundaries: core k block b covers dsts
    # [k*2500 + 128b, min(k*2500 + 128(b+1), (k+1)*2500))
    starts = np.empty(NCORES * NBLK, np.int64)
    ends = np.empty(NCORES * NBLK, np.int64)
    for k in range(NCORES):
        for b in range(NBLK):
            lo = min(k * NPC + 128 * b, (k + 1) * NPC)
            hi = min(k * NPC + 128 * (b + 1), (k + 1) * NPC)
            starts[k * NBLK + b] = np.searchsorted(dst_s, lo)
            ends[k * NBLK + b] = np.searchsorted(dst_s, hi)
    counts = ends - starts
    cchunks = max(1, int(np.ceil(counts.max() / 128)))
    L = cchunks * 128  # padded edge slots per block

    # node-major bf16 gather source for layer 1: [NPAD, 128]
    x_nm = np.zeros((NPAD, 128), BF)
    for k in range(NCORES):
        x_nm[k * NPCP : k * NPCP + NPC, :F_IN] = x[k * NPC : (k + 1) * NPC]

    iota = np.tile(np.arange(128, dtype=np.float32), (128, 1))
    ident = np.eye(128, dtype=np.float32)
    ones_row = np.ones((1, 128), np.float32)

    in_maps = []
    for k in range(NCORES):
        src_pad = np.zeros((NBLK, L), np.int64)
        dstrel_pad = np.full((NBLK, L), -1.0, np.float32)
        for b in range(NBLK):
            g = k * NBLK + b
            s, e = starts[g], ends[g]
            n = e - s
            if n:
                src_pad[b, :n] = src_m[s:e]
                dstrel_pad[b, :n] = (
                    dst_s[s:e] - (k * NPC + 128 * b)).astype(np.float32)
        # dma_gather idx layout: idx i at [i % 16, i // 16], replicated
        # across the 8 groups of 16 partitions.
        idx16 = src_pad.reshape(NBLK, L // 16, 16).transpose(0, 2, 1)
        idx_t = np.concatenate(list(np.tile(idx16, (1, 8, 1))), axis=1)
        idx_t = idx_t.astype(np.int16)
        # dstrel layout: edge e = c*128 + p at [p, c]; bf16 (values exact)
        dr = dstrel_pad.reshape(NBLK, cchunks, 128).transpose(0, 2, 1)
        dr_t = np.ascontiguousarray(np.concatenate(list(dr), axis=1))

        # graph id per local padded node, [128, NBLK]; -1 for pad nodes
        flat = np.full(NPCP, -1.0, np.float32)
        flat[:NPC] = batch[k * NPC : (k + 1) * NPC].astype(np.float32)
        batchrel_t = np.ascontiguousarray(flat.reshape(NBLK, 128).T)

        xT = np.zeros((F_IN, NPCP), np.float32)
        xT[:, :NPC] = x[k * NPC : (k + 1) * NPC].T

        in_maps.append(
            {
                "x_nm": x_nm,
                "idx_t": idx_t,
                "dstrel_t": dr_t,
                "batchrel_t": batchrel_t,
                "xTloc_t": np.ascontiguousarray(xT),
                "iota_t": iota,
                "iotabf_t": iota.astype(BF),
                "ident_t": ident,
                "ones_t": ones_row,
            }
        )
    return in_maps, cchunks


def _build_program(cchunks):
    L = cchunks * 128
    nc = bacc.Bacc("TRN2", target_bir_lowering=False, debug=False,
                   num_devices=NCORES)

    x_nm = nc.dram_tensor("x_nm", [NPAD, 128], BF16, kind="ExternalInput")
    idx_t = nc.dram_tensor("idx_t", [128, NBLK * L // 16], mybir.dt.int16,
                           kind="ExternalInput")
    dstrel_t = nc.dram_tensor("dstrel_t", [128, NBLK * cchunks], F32,
                              kind="ExternalInput")
    batchrel_t = nc.dram_tensor("batchrel_t", [128, NBLK], F32,
                                kind="ExternalInput")
    xTloc_t = nc.dram_tensor("xTloc_t", [F_IN, NPCP], F32,
                             kind="ExternalInput")
    iota_t = nc.dram_tensor("iota_t", [128, 128], F32, kind="ExternalInput")
    iotabf_t = nc.dram_tensor("iotabf_t", [128, 128], BF16,
                              kind="ExternalInput")
    ident_t = nc.dram_tensor("ident_t", [128, 128], F32, kind="ExternalInput")
    ones_t = nc.dram_tensor("ones_t", [1, 128], F32, kind="ExternalInput")
    w_rel_in = [nc.dram_tensor(f"w{i}_rel", [F_IN if i == 1 else H, H], F32,
                               kind="ExternalInput") for i in (1, 2, 3)]
    w_root_in = [nc.dram_tensor(f"w{i}_root", [F_IN if i == 1 else H, H], F32,
                                kind="ExternalInput") for i in (1, 2, 3)]
    b_in = [nc.dram_tensor(f"b{i}", [H, 1], F32, kind="ExternalInput")
            for i in (1, 2, 3)]
    w_out_in = nc.dram_tensor("w_out", [H, C_OUT], F32, kind="ExternalInput")
    b_out_in = nc.dram_tensor("b_out", [1, C_OUT], F32, kind="ExternalInput")
    out_t = nc.dram_tensor("out", [G, C_OUT], F32, kind="ExternalOutput")

    with tile.TileContext(nc) as tc, ExitStack() as ctx:
        const = ctx.enter_context(tc.tile_pool(name="const", bufs=1))
        feat = ctx.enter_context(tc.tile_pool(name="feat", bufs=1))
        xe_pool = ctx.enter_context(tc.tile_pool(name="xe", bufs=3))
        m_pool = ctx.enter_context(tc.tile_pool(name="m", bufs=4))
        nm_pool = ctx.enter_context(tc.tile_pool(name="nm", bufs=3))
        sm_pool = ctx.enter_context(tc.tile_pool(name="sm", bufs=1))
        psA = ctx.enter_context(tc.tile_pool(name="psA", bufs=2, space="PSUM"))
        psB = ctx.enter_context(tc.tile_pool(name="psB", bufs=2, space="PSUM"))
        psT = ctx.enter_context(tc.tile_pool(name="psT", bufs=2, space="PSUM"))
        psP = ctx.enter_context(tc.tile_pool(name="psP", bufs=1, space="PSUM"))
        dram = ctx.enter_context(tc.tile_pool(name="dram", bufs=1, space="DRAM"))

        nc.gpsimd.load_library(library_config.mlp)

        idx_sb = const.tile([128, NBLK * L // 16], mybir.dt.int16)
        nc.sync.dma_start(idx_sb[:], idx_t[:])
        dstrel_sb = const.tile([128, NBLK * cchunks], F32)
        nc.sync.dma_start(dstrel_sb[:], dstrel_t[:])
        batchrel_sb = const.tile([128, NBLK], F32)
        nc.sync.dma_start(batchrel_sb[:], batchrel_t[:])
        iota_sb = const.tile([128, 128], F32)
        nc.sync.dma_start(iota_sb[:], iota_t[:])
        iotabf_sb = const.tile([128, 128], BF16)
        nc.sync.dma_start(iotabf_sb[:], iotabf_t[:])
        ident_sb = const.tile([128, 128], F32)
        nc.sync.dma_start(ident_sb[:], ident_t[:])
        ones_sb = const.tile([1, 128], F32)
        nc.sync.dma_start(ones_sb[:], ones_t[:])
        xTloc_sb = const.tile([F_IN, NPCP], F32)
        nc.sync.dma_start(xTloc_sb[:], xTloc_t[:])

        w_rel_sb, w_root_sb, b_sb = [], [], []
        for i in range(3):
            fi = F_IN if i == 0 else H
            wr32 = const.tile([fi, H], F32, name=f"wrel32_{i}")
            nc.sync.dma_start(wr32[:], w_rel_in[i][:])
            wr = const.tile([fi, H], BF16, name=f"wrel{i}")
            nc.scalar.copy(wr[:], wr32[:])
            w_rel_sb.append(wr)
            wo = const.tile([fi, H], F32, name=f"wroot{i}")
            nc.sync.dma_start(wo[:], w_root_in[i][:])
            w_root_sb.append(wo)
            bb = const.tile([H, 1], F32, name=f"b{i}")
            nc.sync.dma_start(bb[:], b_in[i][:])
            b_sb.append(bb)
        wout_sb = const.tile([H, C_OUT], F32)
        nc.sync.dma_start(wout_sb[:], w_out_in[:])
        bout_sb = const.tile([1, C_OUT], F32)
        nc.sync.dma_start(bout_sb[:], b_out_in[:])

        h1T_sb = feat.tile([H, NPCP], F32)
        h2T_sb = feat.tile([H, NPCP], F32)
        h3T_sb = feat.tile([H, NPCP], F32)
        agg_sb = [feat.tile([128, NPCP], BF16, name=f"agg{i}")
                  for i in range(3)]

        h_loc = [dram.tile([NPCP, H], BF16, name=f"hloc{i}")
                 for i in range(2)]
        h_full = [dram.tile([NPAD, H], BF16, name=f"hfull{i}",
                            addr_space="Shared") for i in range(2)]
        pool_in = dram.tile([H, G], F32)
        pool_out = dram.tile([H, G], F32)

        def gcn_layer(li, f_in, gather_src, rootT_sb, outT_sb):
            wrel, wroot, bb = w_rel_sb[li], w_root_sb[li], b_sb[li]
            aggT = agg_sb[li]
            for b in range(NBLK):
                xe = xe_pool.tile([128, cchunks * 128], BF16, tag="xe",
                                  name=f"xe{li}_{b}")
                GCH = 8  # chunks per dma_gather call (1024 idx max)
                for g0 in range(0, cchunks, GCH):
                    g1 = min(g0 + GCH, cchunks)
                    nsub = (g1 - g0) * 128
                    xe3 = xe[:, g0 * 128 : g1 * 128].rearrange(
                        "p (c f) -> p c f", f=128)
                    nc.gpsimd.dma_gather(
                        xe3, gather_src[:],
                        idx_sb[:, b * (L // 16) + g0 * 8
                               : b * (L // 16) + g0 * 8 + nsub // 16],
                        nsub, nsub, 128)
                agg_ps = psA.tile([128, 128], F32, tag="agg",
                                  name=f"agg{li}_{b}")
                for c in range(cchunks):
                    m = m_pool.tile([128, 128], BF16, tag="m",
                                    name=f"m{li}_{b}_{c}")
                    nc.vector.tensor_scalar(
                        m[:], iotabf_sb[:],
                        dstrel_sb[:, b * cchunks + c : b * cchunks + c + 1],
                        None, ALU.is_equal)
                    nc.tensor.matmul(
                        agg_ps[:f_in, :],
                        xe[:, c * 128 : c * 128 + f_in],
                        m[:],
                        start=(c == 0), stop=(c == cchunks - 1))
                nc.scalar.copy(
                    aggT[:f_in, b * 128 : (b + 1) * 128], agg_ps[:f_in, :])
            # dense transform + bias + relu (feature-major)
            for g in range(NPCP // 512):
                hp = psB.tile([H, 512], F32, tag="hp", name=f"hp{li}_{g}")
                nc.tensor.matmul(hp[:], wrel[:],
                                 aggT[:f_in, g * 512 : (g + 1) * 512],
                                 start=True, stop=False)
                nc.tensor.matmul(hp[:], wroot[:],
                                 rootT_sb[:f_in, g * 512 : (g + 1) * 512],
                                 start=False, stop=True)
                nc.scalar.activation(outT_sb[:, g * 512 : (g + 1) * 512],
                                     hp[:], AF.Relu, bias=bb[:])
            # node-major bf16 store + allgather for next layer's gather source
            if li < 2:
                for b in range(NBLK):
                    tp = psT.tile([128, 128], F32, tag="tp",
                                  name=f"tp{li}_{b}")
                    nc.tensor.transpose(
                        tp[:], outT_sb[:, b * 128 : (b + 1) * 128],
                        ident_sb[:])
                    nm = nm_pool.tile([128, 128], BF16, tag="nm",
                                      name=f"nm{li}_{b}")
                    nc.scalar.copy(nm[:], tp[:])
                    nc.sync.dma_start(h_loc[li][b * 128 : (b + 1) * 128, :],
                                      nm[:])
                nc.gpsimd.collective_compute(
                    "AllGather", ALU.bypass,
                    replica_groups=[list(range(NCORES))],
                    ins=[h_loc[li].opt()], outs=[h_full[li].opt()])

        gcn_layer(0, F_IN, x_nm, xTloc_sb, h1T_sb)
        gcn_layer(1, H, h_full[0], h1T_sb, h2T_sb)
        gcn_layer(2, H, h_full[1], h2T_sb, h3T_sb)

        # ---- pooling: pooledT[h, g] = sum_n h3[n, h] * (batch[n] == g) ----
        pool_ps = psP.tile([H, G], F32)
        for b in range(NBLK):
            tp = psT.tile([128, 128], F32, tag="tp", name=f"tpp_{b}")
            nc.tensor.transpose(tp[:], h3T_sb[:, b * 128 : (b + 1) * 128],
                                ident_sb[:])
            nm = nm_pool.tile([128, 128], F32, tag="nmp", name=f"nmp_{b}")
            nc.scalar.copy(nm[:], tp[:])
            pb = m_pool.tile([128, 128], F32, tag="pb", name=f"pb_{b}")
            nc.vector.tensor_scalar(pb[:], iota_sb[:],
                                    batchrel_sb[:, b : b + 1], None,
                                    ALU.is_equal)
            nc.tensor.matmul(pool_ps[:], nm[:], pb[:],
                             start=(b == 0), stop=(b == NBLK - 1))
        poolT_sb = sm_pool.tile([H, G], F32)
        nc.vector.tensor_copy(poolT_sb[:], pool_ps[:])
        nc.sync.dma_start(pool_in[:], poolT_sb[:])
        nc.gpsimd.collective_compute(
            "AllReduce", ALU.add, replica_groups=[list(range(NCORES))],
            ins=[pool_in.opt()], outs=[pool_out.opt()])
        poolT_full = sm_pool.tile([H, G], F32)
        nc.sync.dma_start(poolT_full[:], pool_out[:])

        # ---- logits = pooled @ w_out + b_out, then log_softmax ----
        log_ps = psB.tile([H, 512], F32, tag="hp", name="log_ps")
        nc.tensor.matmul(log_ps[:G, :C_OUT], poolT_full[:], wout_sb[:],
                         start=True, stop=False)
        nc.tensor.matmul(log_ps[:G, :C_OUT], ones_sb[:], bout_sb[:],
                         start=False, stop=True)
        logits = sm_pool.tile([G, C_OUT], F32)
        nc.vector.tensor_copy(logits[:], log_ps[:G, :C_OUT])
        mx = sm_pool.tile([G, 1], F32)
        nc.vector.tensor_reduce(mx[:], logits[:], mybir.AxisListType.X,
                                ALU.max)
        negmx = sm_pool.tile([G, 1], F32)
        nc.scalar.mul(negmx[:], mx[:], -1.0)
        expv = sm_pool.tile([G, C_OUT], F32)
        nc.scalar.activation(expv[:], logits[:], AF.Exp, bias=negmx[:])
        sm = sm_pool.tile([G, 1], F32)
        nc.vector.tensor_reduce(sm[:], expv[:], mybir.AxisListType.X, ALU.add)
        lse = sm_pool.tile([G, 1], F32)
        nc.scalar.activation(lse[:], sm[:], AF.Ln)
        mxlse = sm_pool.tile([G, 1], F32)
        nc.vector.tensor_add(mxlse[:], mx[:], lse[:])
        outv = sm_pool.tile([G, C_OUT], F32)
        nc.vector.tensor_scalar(outv[:], logits[:], mxlse[:], None,
                                ALU.subtract)
        nc.sync.dma_start(out_t[:], outv[:])

    nc.compile()
    return nc


_CACHE = {}


def kernel(x, edge_index, batch, w1_rel, b1, w1_root, w2_rel, b2, w2_root,
           w3_rel, b3, w3_root, w_out, b_out):
    in_maps, cchunks = _prep_inputs(x, edge_index, batch)
    weights = {
        "w1_rel": np.asarray(w1_rel, np.float32),
        "w1_root": np.asarray(w1_root, np.float32),
        "w2_rel": np.asarray(w2_rel, np.float32),
        "w2_root": np.asarray(w2_root, np.float32),
        "w3_rel": np.asarray(w3_rel, np.float32),
        "w3_root": np.asarray(w3_root, np.float32),
        "b1": np.asarray(b1, np.float32).reshape(H, 1),
        "b2": np.asarray(b2, np.float32).reshape(H, 1),
        "b3": np.asarray(b3, np.float32).reshape(H, 1),
        "w_out": np.asarray(w_out, np.float32),
        "b_out": np.asarray(b_out, np.float32).reshape(1, C_OUT),
    }
    for m in in_maps:
        m.update(weights)

    if cchunks not in _CACHE:
        _CACHE[cchunks] = _build_program(cchunks)
    nc = _CACHE[cchunks]
    res = run_bass_kernel_spmd(nc, in_maps, core_ids=list(range(NCORES)))
    return np.asarray(res.results[0]["out"], np.float32)


# revision 23
# speedup vs baseline: 3.1363x; 2.1485x over previous
"""Distributed 3-layer GraphConv GNN on 8 Trainium2 NeuronCores.

Sharding: nodes (and their incoming edges) are partitioned contiguously
across the 8 cores (2500 real nodes / 20 blocks of 128 padded dst rows per
core). The gather source (node features) lives in DRAM node-major in bf16
(gather cost is descriptor-bound, so bf16 halves SBUF/compute, not DMA).
Per layer, each core:
  - dma_gathers the source-node bf16 feature rows for its edges (sorted by
    dst, padded to a uniform chunk count so the SPMD program is identical
    on every core),
  - segment-sums them per 128-dst block on the TensorEngine via one-hot
    matmuls in bf16 (one-hot built on the VectorEngine with is_equal),
  - applies the dense W_rel (bf16) / W_root (fp32, reads the previous
    layer's resident feature-major activations) transform + bias + ReLU,
  - transposes its local output to node-major bf16 and AllGathers into the
    full gather-source matrix for the next layer (Shared DRAM output).
Graph pooling is a segment-sum over the (sorted) batch vector via the same
one-hot matmul trick in fp32, AllReduced across cores, followed by the
output projection and log_softmax (computed redundantly per core).
"""
import sys

sys.path.insert(0, "/opt/trn_rl_repo")

from contextlib import ExitStack

import numpy as np
import ml_dtypes

import concourse.bass as bass
import concourse.tile as tile
from concourse import bacc, mybir, library_config
from concourse.bass_utils import run_bass_kernel_spmd

N, E, F_IN, H, C_OUT, G = 20000, 640000, 64, 128, 10, 128
NCORES = 8
NPC = 2500            # real nodes per core
NBLK = 20             # dst blocks of 128 per core
NPCP = NBLK * 128     # padded rows per core (2560)
NPAD = NCORES * NPCP  # padded total rows (20480)
F32 = mybir.dt.float32
BF16 = mybir.dt.bfloat16
AF = mybir.ActivationFunctionType
ALU = mybir.AluOpType
BF = ml_dtypes.bfloat16


def _prep_inputs(x, edge_index, batch):
    """Host-side edge partitioning/padding. Returns per-core input dicts
    (minus weights) and the uniform chunks-per-block count."""
    src = np.asarray(edge_index[0], dtype=np.int64)
    dst = np.asarray(edge_index[1], dtype=np.int64)
    batch = np.asarray(batch, dtype=np.int64)
    x = np.ascontiguousarray(np.asarray(x, dtype=np.float32))

    order = np.argsort(dst, kind="stable")
    dst_s = dst[order]
    # remap src to padded node-major rows: core k slab at [k*2560, k*2560+2500)
    src_s = src[order]
    src_m = (src_s // NPC) * NPCP + (src_s % NPC)

    # per-core-block boundaries: core k block b covers dsts
    # [k*2500 + 128b, min(k*2500 + 128(b+1), (k+1)*2500))
    starts = np.empty(NCORES * NBLK, np.int64)
    ends = np.empty(NCORES * NBLK, np.int64)
    for k in range(NCORES):
        for b in range(NBLK):
            lo = min(k * NPC + 128 * b, (k + 1) * NPC)
            hi = min(k * NPC + 128 * (b + 1), (k + 1) * NPC)
            starts[k * NBLK + b] = np.searchsorted(dst_s, lo)
            ends[k * NBLK + b] = np.searchsorted(dst_s, hi)
    counts = ends - starts
    cchunks = max(1, int(np.ceil(counts.max() / 128)))
    L = cchunks * 128  # padded edge slots per block

    # node-major bf16 gather source for layer 1: [NPAD, 128]
    x_nm = np.zeros((NPAD, 128), BF)
    for k in range(NCORES):
        x_nm[k * NPCP : k * NPCP + NPC, :F_IN] = x[k * NPC : (k + 1) * NPC]

    iota = np.tile(np.arange(128, dtype=np.float32), (128, 1))
    ident = np.eye(128, dtype=np.float32)
    ones_row = np.ones((1, 128), np.float32)

    in_maps = []
    for k in range(NCORES):
        # pad slots gather row 0 (negative "skip" indices wedge the device;
        # the zero one-hot columns cancel the padded rows downstream).
        src_pad = np.full((NBLK, L), 0, np.int64)
        dstrel_pad = np.full((NBLK, L), -1.0, np.float32)
        for b in range(NBLK):
            g = k * NBLK + b
            s, e = starts[g], ends[g]
            n = e - s
            if n:
                src_pad[b, :n] = src_m[s:e]
                dstrel_pad[b, :n] = (
                    dst_s[s:e] - (k * NPC + 128 * b)).astype(np.float32)
        # dma_gather idx layout: idx i at [i % 16, i // 16], replicated
        # across the 8 groups of 16 partitions.
        idx16 = src_pad.reshape(NBLK, L // 16, 16).transpose(0, 2, 1)
        idx_t = np.concatenate(list(np.tile(idx16, (1, 8, 1))), axis=1)
        idx_t = idx_t.astype(np.int16)
        # dstrel layout: edge e = c*128 + p at [p, c]; bf16 (values exact)
        dr = dstrel_pad.reshape(NBLK, cchunks, 128).transpose(0, 2, 1)
        dr_t = np.ascontiguousarray(np.concatenate(list(dr), axis=1))

        # graph id per local padded node, [128, NBLK]; -1 for pad nodes
        flat = np.full(NPCP, -1.0, np.float32)
        flat[:NPC] = batch[k * NPC : (k + 1) * NPC].astype(np.float32)
        batchrel_t = np.ascontiguousarray(flat.reshape(NBLK, 128).T)

        xT = np.zeros((F_IN, NPCP), np.float32)
        xT[:, :NPC] = x[k * NPC : (k + 1) * NPC].T

        in_maps.append(
            {
                "x_nm": x_nm,
                "idx_t": idx_t,
                "dstrel_t": dr_t,
                "batchrel_t": batchrel_t,
                "xTloc_t": np.ascontiguousarray(xT),
                "iota_t": iota,
                "iotabf_t": iota.astype(BF),
                "ident_t": ident,
                "ones_t": ones_row,
            }
        )
    return in_maps, cchunks


def _build_program(cchunks):
    L = cchunks * 128
    nc = bacc.Bacc("TRN2", target_bir_lowering=False, debug=False,
                   num_devices=NCORES, num_swdge_queues=4)

    x_nm = nc.dram_tensor("x_nm", [NPAD, 128], BF16, kind="ExternalInput")
    idx_t = nc.dram_tensor("idx_t", [128, NBLK * L // 16], mybir.dt.int16,
                           kind="ExternalInput")
    dstrel_t = nc.dram_tensor("dstrel_t", [128, NBLK * cchunks], F32,
                              kind="ExternalInput")
    batchrel_t = nc.dram_tensor("batchrel_t", [128, NBLK], F32,
                                kind="ExternalInput")
    xTloc_t = nc.dram_tensor("xTloc_t", [F_IN, NPCP], F32,
                             kind="ExternalInput")
    iota_t = nc.dram_tensor("iota_t", [128, 128], F32, kind="ExternalInput")
    iotabf_t = nc.dram_tensor("iotabf_t", [128, 128], BF16,
                              kind="ExternalInput")
    ident_t = nc.dram_tensor("ident_t", [128, 128], F32, kind="ExternalInput")
    ones_t = nc.dram_tensor("ones_t", [1, 128], F32, kind="ExternalInput")
    w_rel_in = [nc.dram_tensor(f"w{i}_rel", [F_IN if i == 1 else H, H], F32,
                               kind="ExternalInput") for i in (1, 2, 3)]
    w_root_in = [nc.dram_tensor(f"w{i}_root", [F_IN if i == 1 else H, H], F32,
                                kind="ExternalInput") for i in (1, 2, 3)]
    b_in = [nc.dram_tensor(f"b{i}", [H, 1], F32, kind="ExternalInput")
            for i in (1, 2, 3)]
    w_out_in = nc.dram_tensor("w_out", [H, C_OUT], F32, kind="ExternalInput")
    b_out_in = nc.dram_tensor("b_out", [1, C_OUT], F32, kind="ExternalInput")
    out_t = nc.dram_tensor("out", [G, C_OUT], F32, kind="ExternalOutput")

    with tile.TileContext(nc) as tc, ExitStack() as ctx:
        const = ctx.enter_context(tc.tile_pool(name="const", bufs=1))
        feat = ctx.enter_context(tc.tile_pool(name="feat", bufs=1))
        xe_pool = ctx.enter_context(tc.tile_pool(name="xe", bufs=6))
        m_pool = ctx.enter_context(tc.tile_pool(name="m", bufs=64))
        pb_pool = ctx.enter_context(tc.tile_pool(name="pb", bufs=4))
        nm_pool = ctx.enter_context(tc.tile_pool(name="nm", bufs=6))
        sm_pool = ctx.enter_context(tc.tile_pool(name="sm", bufs=1))
        psA = ctx.enter_context(tc.tile_pool(name="psA", bufs=2, space="PSUM"))
        psB = ctx.enter_context(tc.tile_pool(name="psB", bufs=2, space="PSUM"))
        psT = ctx.enter_context(tc.tile_pool(name="psT", bufs=2, space="PSUM"))
        psP = ctx.enter_context(tc.tile_pool(name="psP", bufs=1, space="PSUM"))
        dram = ctx.enter_context(tc.tile_pool(name="dram", bufs=1, space="DRAM"))

        nc.gpsimd.load_library(library_config.mlp)

        idx_sb = const.tile([128, NBLK * L // 16], mybir.dt.int16)
        nc.sync.dma_start(idx_sb[:], idx_t[:])
        dstrel_sb = const.tile([128, NBLK * cchunks], F32)
        nc.sync.dma_start(dstrel_sb[:], dstrel_t[:])
        batchrel_sb = const.tile([128, NBLK], F32)
        nc.sync.dma_start(batchrel_sb[:], batchrel_t[:])
        iota_sb = const.tile([128, 128], F32)
        nc.sync.dma_start(iota_sb[:], iota_t[:])
        iotabf_sb = const.tile([128, 128], BF16)
        nc.sync.dma_start(iotabf_sb[:], iotabf_t[:])
        ident_sb = const.tile([128, 128], F32)
        nc.sync.dma_start(ident_sb[:], ident_t[:])
        ones_sb = const.tile([1, 128], F32)
        nc.sync.dma_start(ones_sb[:], ones_t[:])
        xTloc_sb = const.tile([F_IN, NPCP], F32)
        nc.sync.dma_start(xTloc_sb[:], xTloc_t[:])

        w_rel_sb, w_root_sb, b_sb = [], [], []
        for i in range(3):
            fi = F_IN if i == 0 else H
            wr32 = const.tile([fi, H], F32, name=f"wrel32_{i}")
            nc.sync.dma_start(wr32[:], w_rel_in[i][:])
            wr = const.tile([fi, H], BF16, name=f"wrel{i}")
            nc.scalar.copy(wr[:], wr32[:])
            w_rel_sb.append(wr)
            wo = const.tile([fi, H], F32, name=f"wroot{i}")
            nc.sync.dma_start(wo[:], w_root_in[i][:])
            w_root_sb.append(wo)
            bb = const.tile([H, 1], F32, name=f"b{i}")
            nc.sync.dma_start(bb[:], b_in[i][:])
            b_sb.append(bb)
        wout_sb = const.tile([H, C_OUT], F32)
        nc.sync.dma_start(wout_sb[:], w_out_in[:])
        bout_sb = const.tile([1, C_OUT], F32)
        nc.sync.dma_start(bout_sb[:], b_out_in[:])

        h1T_sb = feat.tile([H, NPCP], F32)
        h2T_sb = feat.tile([H, NPCP], F32)
        h3T_sb = feat.tile([H, NPCP], F32)
        agg_sb = [feat.tile([128, NPCP], BF16, name=f"agg{i}")
                  for i in range(3)]

        h_loc = [dram.tile([NPCP, H], BF16, name=f"hloc{i}")
                 for i in range(2)]
        h_full = [dram.tile([NPAD, H], BF16, name=f"hfull{i}",
                            addr_space="Shared") for i in range(2)]
        pool_in = dram.tile([G, C_OUT], F32)
        pool_out = dram.tile([G, C_OUT], F32)

        # gather in groups of GBLK blocks so dma_gather calls are full
        # 1024-idx calls (at most one short tail call per group).
        GBLK = 2
        qc = [0]

        def gcn_layer(li, f_in, gather_src, rootT_sb, outT_sb):
            wrel, wroot, bb = w_rel_sb[li], w_root_sb[li], b_sb[li]
            aggT = agg_sb[li]
            for b0 in range(0, NBLK, GBLK):
                xe = xe_pool.tile([128, GBLK * cchunks * 128], BF16,
                                  tag="xe", name=f"xe{li}_{b0}")
                nchunk = GBLK * cchunks
                for g0 in range(0, nchunk, 8):
                    g1 = min(g0 + 8, nchunk)
                    nsub = (g1 - g0) * 128
                    xe3 = xe[:, g0 * 128 : g1 * 128].rearrange(
                        "p (c f) -> p c f", f=128)
                    nc.gpsimd.dma_gather(
                        xe3, gather_src[:],
                        idx_sb[:, b0 * (L // 16) + g0 * 8
                               : b0 * (L // 16) + g0 * 8 + nsub // 16],
                        nsub, nsub, 128, queue_num=qc[0] % 4)
                    qc[0] += 1
                for b in range(b0, min(b0 + GBLK, NBLK)):
                    boff = (b - b0) * cchunks * 128
                    agg_ps = psA.tile([128, 128], F32, tag="agg",
                                      name=f"agg{li}_{b}")
                    for c in range(cchunks):
                        m = m_pool.tile([128, 128], BF16, tag="m",
                                        name=f"m{li}_{b}_{c}")
                        nc.vector.tensor_scalar(
                            m[:], iotabf_sb[:],
                            dstrel_sb[:, b * cchunks + c
                                      : b * cchunks + c + 1],
                            None, ALU.is_equal)
                        nc.tensor.matmul(
                            agg_ps[:f_in, :],
                            xe[:, boff + c * 128 : boff + c * 128 + f_in],
                            m[:],
                            start=(c == 0), stop=(c == cchunks - 1))
                    nc.scalar.copy(
                        aggT[:f_in, b * 128 : (b + 1) * 128],
                        agg_ps[:f_in, :])
            # dense transform + bias + relu (feature-major)
            for g in range(NPCP // 512):
                hp = psB.tile([H, 512], F32, tag="hp", name=f"hp{li}_{g}")
                nc.tensor.matmul(hp[:], wrel[:],
                                 aggT[:f_in, g * 512 : (g + 1) * 512],
                                 start=True, stop=False)
                nc.tensor.matmul(hp[:], wroot[:],
                                 rootT_sb[:f_in, g * 512 : (g + 1) * 512],
                                 start=False, stop=True)
                nc.scalar.activation(outT_sb[:, g * 512 : (g + 1) * 512],
                                     hp[:], AF.Relu, bias=bb[:])
            # node-major bf16 store + allgather for next layer's gather source
            if li < 2:
                for b in range(NBLK):
                    tp = psT.tile([128, 128], F32, tag="tp",
                                  name=f"tp{li}_{b}")
                    nc.tensor.transpose(
                        tp[:], outT_sb[:, b * 128 : (b + 1) * 128],
                        ident_sb[:])
                    nm = nm_pool.tile([128, 128], BF16, tag="nm",
                                      name=f"nm{li}_{b}")
                    nc.scalar.copy(nm[:], tp[:])
                    nc.sync.dma_start(h_loc[li][b * 128 : (b + 1) * 128, :],
                                      nm[:])
                nc.gpsimd.collective_compute(
                    "AllGather", ALU.bypass,
                    replica_groups=[list(range(NCORES))],
                    ins=[h_loc[li].opt()], outs=[h_full[li].opt()])

        gcn_layer(0, F_IN, x_nm, xTloc_sb, h1T_sb)
        gcn_layer(1, H, h_full[0], h1T_sb, h2T_sb)
        gcn_layer(2, H, h_full[1], h2T_sb, h3T_sb)

        # ---- pooling: pooledT[h, g] = sum_n h3[n, h] * (batch[n] == g) ----
        pool_ps = psP.tile([H, G], F32)
        for b in range(NBLK):
            tp = psT.tile([128, 128], F32, tag="tp", name=f"tpp_{b}")
            nc.tensor.transpose(tp[:], h3T_sb[:, b * 128 : (b + 1) * 128],
                                ident_sb[:])
            nm = pb_pool.tile([128, 128], F32, tag="nmp", name=f"nmp_{b}")
            nc.scalar.copy(nm[:], tp[:])
            pb = pb_pool.tile([128, 128], F32, tag="pb", name=f"pb_{b}")
            nc.vector.tensor_scalar(pb[:], iota_sb[:],
                                    batchrel_sb[:, b : b + 1], None,
                                    ALU.is_equal)
            nc.tensor.matmul(pool_ps[:], nm[:], pb[:],
                             start=(b == 0), stop=(b == NBLK - 1))
        poolT_sb = sm_pool.tile([H, G], F32)
        nc.vector.tensor_copy(poolT_sb[:], pool_ps[:])
        # local partial logits [G, C] = pooled_localT^T @ w_out, AllReduced
        # across cores (5KB payload instead of 64KB), bias added after.
        lg_ps = psB.tile([H, 512], F32, tag="hp", name="lg_ps")
        nc.tensor.matmul(lg_ps[:G, :C_OUT], poolT_sb[:], wout_sb[:],
                         start=True, stop=True)
        lgpart = sm_pool.tile([G, C_OUT], F32)
        nc.vector.tensor_copy(lgpart[:], lg_ps[:G, :C_OUT])
        nc.sync.dma_start(pool_in[:], lgpart[:])
        nc.gpsimd.collective_compute(
            "AllReduce", ALU.add, replica_groups=[list(range(NCORES))],
            ins=[pool_in.opt()], outs=[pool_out.opt()])
        lsum = sm_pool.tile([G, C_OUT], F32)
        nc.sync.dma_start(lsum[:], pool_out[:])
        bias_ps = psB.tile([H, 512], F32, tag="hp", name="bias_ps")
        nc.tensor.matmul(bias_ps[:G, :C_OUT], ones_sb[:], bout_sb[:],
                         start=True, stop=True)
        logits = sm_pool.tile([G, C_OUT], F32)
        nc.vector.tensor_add(logits[:], lsum[:], bias_ps[:G, :C_OUT])
        mx = sm_pool.tile([G, 1], F32)
        nc.vector.tensor_reduce(mx[:], logits[:], mybir.AxisListType.X,
                                ALU.max)
        negmx = sm_pool.tile([G, 1], F32)
        nc.scalar.mul(negmx[:], mx[:], -1.0)
        expv = sm_pool.tile([G, C_OUT], F32)
        nc.scalar.activation(expv[:], logits[:], AF.Exp, bias=negmx[:])
        sm = sm_pool.tile([G, 1], F32)
        nc.vector.tensor_reduce(sm[:], expv[:], mybir.AxisListType.X, ALU.add)
        lse = sm_pool.tile([G, 1], F32)
        nc.scalar.activation(lse[:], sm[:], AF.Ln)
        mxlse = sm_pool.tile([G, 1], F32)
        nc.vector.tensor_add(mxlse[:], mx[:], lse[:])
        outv = sm_pool.tile([G, C_OUT], F32)
        nc.vector.tensor_scalar(outv[:], logits[:], mxlse[:], None,
                                ALU.subtract)
        nc.sync.dma_start(out_t[:], outv[:])

    nc.compile()
    return nc


_CACHE = {}


def kernel(x, edge_index, batch, w1_rel, b1, w1_root, w2_rel, b2, w2_root,
           w3_rel, b3, w3_root, w_out, b_out):
    in_maps, cchunks = _prep_inputs(x, edge_index, batch)
    weights = {
        "w1_rel": np.asarray(w1_rel, np.float32),
        "w1_root": np.asarray(w1_root, np.float32),
        "w2_rel": np.asarray(w2_rel, np.float32),
        "w2_root": np.asarray(w2_root, np.float32),
        "w3_rel": np.asarray(w3_rel, np.float32),
        "w3_root": np.asarray(w3_root, np.float32),
        "b1": np.asarray(b1, np.float32).reshape(H, 1),
        "b2": np.asarray(b2, np.float32).reshape(H, 1),
        "b3": np.asarray(b3, np.float32).reshape(H, 1),
        "w_out": np.asarray(w_out, np.float32),
        "b_out": np.asarray(b_out, np.float32).reshape(1, C_OUT),
    }
    for m in in_maps:
        m.update(weights)

    if cchunks not in _CACHE:
        _CACHE[cchunks] = _build_program(cchunks)
    nc = _CACHE[cchunks]
    res = run_bass_kernel_spmd(nc, in_maps, core_ids=list(range(NCORES)))
    return np.asarray(res.results[0]["out"], np.float32)
